# revision 27
# baseline (speedup 1.0000x reference)
"""Multi-head causal attention (B=4, S=2048, D=1024, H=16) on 8 NeuronCores.

Sharding: core c handles batch b = c//2 and head-group g = c%2 (8 heads).
Each core computes QKV projections for its group, causal attention for its
8 heads, and a partial output projection (row-split Wo).  Host sums the two
fp16 partials per batch and adds bo.

Engines execute in-order, so the emission IS the schedule: the attention
t-loop is ACT-bound (exp ~1.04us/t vs ~0.5us/t of PE work), so projection
work is decomposed into "filler" units woven ~2 per t-step into the
attention stream, qk(t+1) is emitted before pv(t) so the wait on exp(t)
lands late, each slice's normalization is deferred into the next slice's
stream, and slices run j-major so output-projection tiles release early
enough to feed later phases.

QKV projections run in fp8e4 DoubleRow (2 k-tiles per matmul, 0.5
cycles/row) with a 3-term / 2-copy residual expansion that keeps
end-to-end error ~1e-3: X@W ~ X8@W8 + R8@W8 + X8@S8 where X8=q8(X),
R8=q8(X-X8) (unscaled: the residual sits just above fp8's subnormal
floor, so it pairs directly with W8), W8=q8(32W), S8=q8(32W-W8).  Only
two weight copies stream through the startup-critical DMA window.  All
terms land at PSUM scale 32;
the Q/K scale is folded into exp's scale argument (0.125/1024) and the V
scale into a 32.0-valued rowsum ones column, so no extra on-chip ops are
spent on rescaling.  Host packs operands in the exact SBUF layouts
[128, d2, kt, copy, n] so every DMA is a contiguous 3-dim transfer.

Attention per pair-tile (heads 2c, 2c+1 on partitions 0-63/64-127):
scoresT tiles [sk=128, sq<=512] via two K=64 fp16 matmuls; windowed
matmuls skip fully-masked regions; diagonal 128x128 blocks masked by a
0/1 tensor_mul on the exp output.  PV is TRANSPOSED: stationary = dense
et chunk [sk=128, sq=128], moving = V [sk=128, 65] (64 cols + ones), so
each accumulation costs 65 moving columns instead of 512; rowsums land at
pv col 64 on their own query's partition.  PSUM zeroing is armed per 2KB
bank, so only each bank's first matmul sets start (skip_group_check).
Normalization: one reciprocal [128, 2, 4] + one broadcast tensor_mul per
(pair, slice) into an fp16 stage [sq, headA|headB]; PE transposes flip
each 128-chunk into [hd-pair, sq] PSUM; one DVE copy lands ot_sb - the
stationary layout the fp16 output projection needs.  The final slice
pre-accumulates cc=0..2 and fine-grains pair 3's evacuation per 128-chunk
to shorten the tail.

Walrus wait-slot legality (1 sem wait per ACT/DVE/DMA instruction): touch
ops pre-observe constant DMAs and a legalization pass splits excess waits
onto same-engine NoOps.
"""

import sys

for _p in ("/opt/trn_rl_repo",):
    if _p not in sys.path:
        sys.path.insert(0, _p)

from collections import deque
from contextlib import ExitStack

import numpy as np
import ml_dtypes

import concourse.bass as bass
import concourse.mybir as mybir
import concourse.tile as tile
from concourse.bass_utils import run_bass_kernel_spmd

import bass_rust

F16 = mybir.dt.float16
F32 = mybir.dt.float32
F8 = mybir.dt.float8e4
DR = mybir.MatmulPerfMode.DoubleRow
TERMS = ((0, 0), (0, 1), (1, 0))  # (w-copy, x-copy): W8*X8 + W8*R8 + S8*X8
QK_TERMS = ((0, 0), (0, 1))  # Q/K drop the S8 term: their fp8 requantization
# noise (~2.4%) dominates the ~0.2% W-residual correction anyway.
AF = mybir.ActivationFunctionType

B, S, D, H = 4, 2048, 1024, 16
HD = D // H  # 64
GH = 8  # heads per group
GW = GH * HD  # 512 columns per group


_SPLITTABLE = {
    "InstMatmult", "InstLdweights", "InstActivation", "InstTensorCopy",
    "InstTensorTensor", "InstTensorScalarPtr", "InstTensorReduce",
    "InstMemset", "InstDMACopy", "InstReciprocal", "InstIota",
    "InstTensorTensorReduce", "InstBNStats", "InstBNStatsAggregate",
    "InstStreamShuffle", "InstNoOp", "InstPool", "InstMax", "InstDrain",
}


def _legalize_waits(nc, max_waits=1):
    """Walrus codegen accepts at most one sync-wait command per engine
    instruction; Tile's wait assigner can emit more.  Split extras onto
    same-engine NoOps inserted immediately before (semantics preserved:
    the engine blocks at the same program point)."""
    ctr = 0
    for fn in nc.m.functions:
        for blk in fn.blocks:
            out = []
            for ins in blk.instructions:
                si = ins.sync_info
                if (
                    si is not None
                    and len(si.on_wait) > max_waits
                    and type(ins).__name__ in _SPLITTABLE
                ):
                    waits = list(si.on_wait)
                    extra, keep = waits[:-max_waits], waits[-max_waits:]
                    for w in extra:
                        nop = mybir.InstNoOp(name=f"waitnop-{ctr}", ins=[], outs=[])
                        ctr += 1
                        nop.engine = ins.engine
                        nop.sync_info = bass_rust.SyncInfo(on_wait=[w], on_update=[])
                        out.append(nop)
                    ins.sync_info = bass_rust.SyncInfo(
                        on_wait=keep, on_update=list(si.on_update)
                    )
                out.append(ins)
            blk.instructions[:] = out
    return ctr


def build_nc(s=S, legalize=True, pump_extra=(500, 100, 0, 0)):
    ns = s // 512  # 512-wide sq slices per head
    nt = s // 128  # 128-wide k tiles
    nd = D // 128  # contraction chunks for projections
    nb = s // 512  # xt column blocks

    nc = bass.Bass("TRN2", target_bir_lowering=False, debug=False)
    xt_d = nc.dram_tensor("xt", [128, 4, 2, 2, s], F8, kind="ExternalInput").ap()
    # wq/wk: W8 only (2-term), c-major so the startup-critical c0 block is
    # one small contiguous transfer: [p, c, d2, kt, n]
    wq_d = nc.dram_tensor("wq", [128, 4, 4, 2, 128], F8, kind="ExternalInput").ap()
    wk_d = nc.dram_tensor("wk", [128, 4, 4, 2, 128], F8, kind="ExternalInput").ap()
    wv_d = nc.dram_tensor("wv", [128, 4, 2, 2, GW], F8, kind="ExternalInput").ap()
    wo_d = nc.dram_tensor("wo", [GW, D], F16, kind="ExternalInput").ap()
    bqk_d = nc.dram_tensor("bqk", [128, 8], F32, kind="ExternalInput").ap()
    bvb_d = nc.dram_tensor("bvb", [128, GW], F16, kind="ExternalInput").ap()
    mask_d = nc.dram_tensor("mask", [128, 128], F16, kind="ExternalInput").ap()
    idn_d = nc.dram_tensor("idn", [128, 128], F16, kind="ExternalInput").ap()
    out_d = nc.dram_tensor("out", [s, D], F16, kind="ExternalOutput").ap()

    with tile.TileContext(nc) as tc, ExitStack() as ctx:
        pool = lambda name, bufs, **kw: ctx.enter_context(
            tc.tile_pool(name=name, bufs=bufs, **kw)
        )
        const_p = pool("const", 1)
        xt_p = pool("xtp", nb)
        w_p = pool("wp", 1)
        qt_p = pool("qtp", 4)
        kt_p = pool("ktp", 4)
        v_p = pool("vp", nt)
        et_p = pool("etp", 6)
        ot_p = pool("otp", 4)
        st_p = pool("stp", 2)
        rc_p = pool("rcp", 2)
        ob_p = pool("obp", 4)
        ps_proj = pool("psproj", 2, space="PSUM")  # [128,512]f32    -> 2 banks
        ps_qk = pool("psqk", 2, space="PSUM")      # [128,2,512]f32  -> 4 banks
        ps_pv = pool("pspv", 1, space="PSUM")      # [128,2,512]f32  -> 2 banks

        # --- input DMAs, ordered by first use.  wv and xt block 0 stream as
        # interleaved per-d pieces so V-proj st0's d-th matmul can fire as
        # soon as its two pieces land. ---
        # fp8 DoubleRow layouts: [partition, double-chunk, k-tile, copy, n]
        wv_sb = w_p.tile([128, 4, 2, 2, GW], F8)
        xt_sb = [
            xt_p.tile([128, 4, 2, 2, 512], F8, tag="xt", name=f"xtb{b}")
            for b in range(nb)
        ]
        bvb_sb = const_p.tile([128, GW], F16)
        wq_sb = w_p.tile([128, 4, 4, 2, 128], F8)
        wk_sb = w_p.tile([128, 4, 4, 2, 128], F8)
        wo_sb = w_p.tile([128, 4, D], F16)
        bqk_sb = const_p.tile([128, 8], F32)
        mask_sb = const_p.tile([128, 128], F16)
        idn_sb = const_p.tile([128, 128], F16)

        # DMA cost lives on the ISSUING engine (~max(500ns, perpart-bytes *
        # 0.39)), so split issue across three otherwise-idle sequencer
        # streams.  SP carries the first-exp critical path (wq/wk c0 block +
        # xt0); DVE the tiny consts needed early; Pool everything else in
        # first-use order.
        nc.sync.dma_start(out=wq_sb[:, 0], in_=wq_d[:, 0])
        nc.sync.dma_start(out=wk_sb[:, 0], in_=wk_d[:, 0])
        nc.sync.dma_start(
            out=xt_sb[0][:], in_=xt_d[:, :, :, :, 0:512]
        )
        nc.sync.dma_start(out=wq_sb[:, 1:4], in_=wq_d[:, 1:4])
        nc.sync.dma_start(out=wk_sb[:, 1:4], in_=wk_d[:, 1:4])
        nc.scalar.dma_start(out=bqk_sb[:], in_=bqk_d[:])
        nc.scalar.dma_start(out=mask_sb[:], in_=mask_d[:])
        nc.gpsimd.dma_start(out=bvb_sb[:], in_=bvb_d[:])
        nc.gpsimd.dma_start(out=idn_sb[:], in_=idn_d[:])
        nc.gpsimd.dma_start(out=wv_sb[:], in_=wv_d[:])
        for bI in range(1, nb):
            nc.gpsimd.dma_start(
                out=xt_sb[bI][:],
                in_=xt_d[:, :, :, :, bI * 512 : (bI + 1) * 512],
            )
        nc.gpsimd.dma_start(out=wo_sb[:], in_=wo_d.rearrange("(c p) n -> p c n", p=128))

        # touch ops: early Exp-table load + const observations
        scr_a = const_p.tile([128, 1], F32)
        nc.scalar.activation(scr_a[:], bqk_sb[:, 0:1], AF.Exp)
        scr_v = const_p.tile([128, 1], F16)
        nc.vector.tensor_copy(scr_v[:], bvb_sb[:, 0:1])
        scr_m = const_p.tile([128, 1], F16)
        nc.vector.tensor_copy(scr_m[:], mask_sb[:, 0:1])
        scr_i = const_p.tile([128, 1], F16)
        nc.vector.tensor_copy(scr_i[:], idn_sb[:, 0:1])

        # Q/K staged in fp8 DoubleRow pair layout [pair-part, slot, seq]:
        # head A pairs on partitions 0-31, head B on 32-63; slot = hd half.
        # The projection's W columns are host-permuted so PSUM partitions
        # land as [A-lo | B-lo | A-hi | B-hi] and two partition-shifted DVE
        # evacs (0:64 and 64:128 -> slot 0/1) build the pairs for free.
        qt_sb = [qt_p.tile([64, 2, s], F8, tag="qt", name=f"qt{c}") for c in range(4)]
        kt_sb = [kt_p.tile([64, 2, s], F8, tag="kt", name=f"kt{c}") for c in range(4)]
        ot_sb = [ot_p.tile([128, s], F16, tag="ot", name=f"ot{c}") for c in range(4)]
        v_sb = [None] * nt

        # ---------- filler generators: one PE matmul (~0.1us) per yield ----
        def gen_vproj(st):
            ps = ps_proj.tile([128, 512], F32, tag="ps", name="ps")
            blk, col = st // 4, (st % 4) * 128
            for i, (d2, (wt, xs)) in enumerate(
                (d2, t) for t in TERMS for d2 in range(4)
            ):
                nc.tensor.matmul(
                    ps[:],
                    xt_sb[blk][:, d2, :, xs, col : col + 128],
                    wv_sb[:, d2, :, wt, :],
                    start=(i == 0), stop=(i == 11), perf_mode=DR,
                )
                if i < 11:
                    yield
            vt = v_p.tile([128, GH, 65], F16, tag="v", name=f"v{st}")
            nc.vector.memset(vt[:, :, 64:65], 32.0)
            nc.vector.tensor_add(
                vt[:, :, 0:64],
                ps[:].rearrange("p (h e) -> p h e", h=GH),
                bvb_sb[:].rearrange("p (h e) -> p h e", h=GH),
            )
            v_sb[st] = vt

        def qk_evac(dst, ps, sl, bcol):
            for half in range(2):
                nc.vector.tensor_scalar_add(
                    dst[0:64, half, sl * 512 : (sl + 1) * 512],
                    ps[64 * half : 64 * half + 64, :],
                    bqk_sb[64 * half : 64 * half + 64, bcol : bcol + 1],
                )

        def gen_qkproj(c, sl):
            for dst, wsb, bcol in ((qt_sb[c], wq_sb, c), (kt_sb[c], wk_sb, 4 + c)):
                ps = ps_proj.tile([128, 512], F32, tag="ps", name="ps")
                for i, (d2, (wt, xs)) in enumerate(
                    (d2, t) for t in QK_TERMS for d2 in range(4)
                ):
                    nc.tensor.matmul(
                        ps[:],
                        wsb[:, c, d2, :, :],
                        xt_sb[sl][:, d2, :, xs, :],
                        start=(i == 0), stop=(i == 7), perf_mode=DR,
                    )
                    if i < 7:
                        yield
                qk_evac(dst, ps, sl, bcol)
                yield

        def out_proj(st, dsl, pp, ptag, ccs, po=None):
            """Accumulate output projection for s-tile st, D-half dsl over
            pair-chunks ccs; evacuate + DMA when 3 in ccs."""
            if po is None:
                po = pp.tile([128, 512], F32, tag=ptag, name="po")
            for cc in ccs:
                nc.tensor.matmul(
                    po[:],
                    ot_sb[cc][:, st * 128 : (st + 1) * 128],
                    wo_sb[:, cc, dsl * 512 : (dsl + 1) * 512],
                    start=(cc == 0),
                    stop=(cc == 3),
                )
            if 3 in ccs:
                ob = ob_p.tile([128, 512], F16, tag="ob", name="ob")
                nc.vector.tensor_copy(ob[:], po[:])
                nc.gpsimd.dma_start(
                    out=out_d[
                        st * 128 : (st + 1) * 128, dsl * 512 : (dsl + 1) * 512
                    ],
                    in_=ob[:],
                )
            return po

        def gen_oproj(j):
            for st in range(4 * j, 4 * j + 4):
                ob = ob_p.tile([128, D], F16, tag="ob", name="ob")
                for dsl in range(2):
                    po = ps_proj.tile([128, 512], F32, tag="ps", name="po")
                    for cc in range(4):
                        nc.tensor.matmul(
                            po[:],
                            ot_sb[cc][:, st * 128 : (st + 1) * 128],
                            wo_sb[:, cc, dsl * 512 : (dsl + 1) * 512],
                            start=(cc == 0),
                            stop=(cc == 3),
                        )
                        if cc < 3:
                            yield
                    nc.vector.tensor_copy(
                        ob[:, dsl * 512 : (dsl + 1) * 512], po[:]
                    )
                    yield
                nc.gpsimd.dma_start(
                    out=out_d[st * 128 : (st + 1) * 128, :], in_=ob[:]
                )

        # Two filler queues, both (deadline, release, gen) FIFO in use-order.
        # high = Q/K/V projections (their outputs gate the attention stream;
        # one unit = one fp8-DR matmul ~107ns); low = output projections
        # (one unit ~213ns fp16, no deadline pressure until the tail) -
        # pumped only when high is empty or release-gated, soaking up the
        # late windows' PE slack.  At most one gen per queue is mid-flight,
        # so ps_proj's two bufs bound the open PSUM groups.
        HI_COST, LO_COST = 107, 213
        high_q = deque()
        low_q = deque()
        cur_slice = [(0, 0)]

        def _pump_one(q):
            # advance q's front gen by one unit if releasable
            while q:
                _, rel, g = q[0]
                if rel > cur_slice[0]:
                    return 0
                try:
                    next(g)
                    return 1
                except StopIteration:
                    q.popleft()
            return 0

        def pump(budget):
            # spend up to ~budget ns of PE time on filler units, high first
            spent = 0
            while spent < budget:
                if _pump_one(high_q):
                    spent += HI_COST
                elif _pump_one(low_q):
                    spent += LO_COST
                else:
                    break
            return spent

        def vwait(t):
            # pv(t) references v_sb[t]; its gen sits in high_q in use-order.
            while v_sb[t] is None:
                before = len(high_q)
                if _pump_one(high_q) == 0 and len(high_q) == before:
                    raise RuntimeError(f"v filler starved for t={t}")

        def flush(cj):
            for q in (high_q, low_q):
                while q and q[0][0] <= cj:
                    _, _, g = q[0]
                    for _ in g:
                        pass
                    q.popleft()

        # startup: Q/K for (c=0, sl=0) emitted inline, d2-interleaved with
        # the arriving wq/wk/xt DMA pieces, so the first exp fires as soon
        # as ~2 MB have streamed in.  V st0-3 follow as fillers pumped from
        # inside slice (0,0)'s own t-loop.
        qps = ps_proj.tile([128, 512], F32, tag="ps", name="ps")
        kps = ps_proj.tile([128, 512], F32, tag="ps", name="ps")
        for ti, (wt, xs) in enumerate(QK_TERMS):
            first, last_t = ti == 0, ti == 1
            for d2 in range(4):
                for ps_t, wsb in ((qps, wq_sb), (kps, wk_sb)):
                    nc.tensor.matmul(
                        ps_t[:],
                        wsb[:, 0, d2, :, :],
                        xt_sb[0][:, d2, :, xs, :],
                        start=(first and d2 == 0), stop=(last_t and d2 == 3),
                        perf_mode=DR,
                    )
        qk_evac(qt_sb[0], qps, 0, 0)
        qk_evac(kt_sb[0], kps, 0, 4)

        # high queue in use-order.  j=0: qkproj c1-3 first (wq/wk/xt0 have
        # landed; slice (0,0) defers all pvs to its end so vwait can't
        # block the cheap qkproj pumps on the wv stream), then V st0-3.
        # Releases approximate DMA landing: xt block 1/2/3 arrive ~12/15/18
        # us, i.e. around slices (0,2)/(0,3)/(1,0).
        for c in range(1, 4):
            high_q.append(((0, c), (0, 0), gen_qkproj(c, 0)))
        for st in range(4):
            high_q.append(((0, 1), (0, 0), gen_vproj(st)))
        xt_rel = {1: (0, 2), 2: (0, 3), 3: (1, 0)}  # ~ DMA landing slices
        for j in range(1, ns):
            rel = xt_rel[j]
            high_q.append(((j, 0), rel, gen_qkproj(0, j)))
            for st in range(4 * j, 4 * j + 4):
                high_q.append(((j, 1), rel, gen_vproj(st)))
            for c in range(1, 4):
                high_q.append(((j, c), rel, gen_qkproj(c, j)))

        def evac_views(pv, stage, rcp, ci=None):
            """APs for the normalization mul over all 4 chunks (ci=None) or a
            single 128-chunk ci."""
            if ci is None:
                pv_v = bass.AP(
                    tensor=pv.tensor, offset=pv.offset,
                    ap=[pv.ap[0], [512, 2], [65, 4], [1, 64]],
                )
                st_v = bass.AP(
                    tensor=stage.tensor, offset=stage.offset,
                    ap=[stage.ap[0], [64, 2], [128, 4], [1, 64]],
                )
                rc_v = bass.AP(
                    tensor=rcp.tensor, offset=rcp.offset,
                    ap=[rcp.ap[0], [4, 2], [1, 4], [0, 64]],
                )
            else:
                pv_v = bass.AP(
                    tensor=pv.tensor, offset=pv.offset + ci * 65,
                    ap=[pv.ap[0], [512, 2], [1, 64]],
                )
                st_v = bass.AP(
                    tensor=stage.tensor, offset=stage.offset + ci * 128,
                    ap=[stage.ap[0], [64, 2], [1, 64]],
                )
                rc_v = bass.AP(
                    tensor=rcp.tensor, offset=rcp.offset + ci,
                    ap=[rcp.ap[0], [4, 2], [0, 64]],
                )
            return st_v, pv_v, rc_v

        prev_evac = [None]

        def emit_prev_evac():
            if prev_evac[0] is not None:
                prev_evac[0]()
                prev_evac[0] = None

        def attention(c, j):
            final = c == 3 and j == ns - 1
            cur_slice[0] = (j, c)
            # pv psum [128, 2, 512]: head hh in its own bank; chunk ci
            # occupies words ci*65..ci*65+65 (never straddles a bank).
            pv = ps_pv.tile([128, 2, 512], F32, tag="pv", name="pv")
            last = 4 * j + 3

            def emit_qk(t):
                diag = t >= 4 * j
                w0 = 128 * (t - 4 * j) if diag else 0
                qk = ps_qk.tile([128, 2, 512], F32, tag="qk", name="qk")
                for hh in range(2):
                    b0 = 32 * hh
                    nc.tensor.matmul(
                        qk[:, hh, w0:512],
                        kt_sb[c][b0 : b0 + 32, :, t * 128 : (t + 1) * 128],
                        qt_sb[c][b0 : b0 + 32, :, j * 512 + w0 : (j + 1) * 512],
                        start=True,
                        stop=True,
                        perf_mode=DR,
                    )
                et = et_p.tile([128, 2, 512], F16, tag="et", name="et")
                nc.scalar.activation(
                    et[:, :, w0:512], qk[:, :, w0:512], AF.Exp, scale=0.125 / 1024.0
                )
                if diag:
                    for hh in range(2):
                        nc.vector.tensor_mul(
                            et[:, hh, w0 : w0 + 128],
                            et[:, hh, w0 : w0 + 128],
                            mask_sb[:],
                        )
                return et, w0

            def emit_pv(t, et, w0):
                # transposed PV: stationary = dense et chunk, moving = V.
                # PSUM zeroing is armed per 2KB bank: only the bank's first
                # matmul (ci=0, t=0) sets start; the other chunks' first
                # writes consume the bank-wide pending-zero.
                for ci in range(w0 // 128, 4):
                    for hh in range(2):
                        nc.tensor.matmul(
                            pv[:, hh, ci * 65 : ci * 65 + 65],
                            et[:, hh, ci * 128 : (ci + 1) * 128],
                            v_sb[t][:, 2 * c + hh, 0:65],
                            start=(t == 0 and ci == 0),
                            stop=(t == last),
                            skip_group_check=True,
                        )

            # Slice (0,0) defers all pv's to the end (lag > last): the wv
            # DMA hasn't landed yet, and vwait would otherwise force the
            # DMA-gated V fillers ahead of the cheap qkproj fillers.
            lag = last + 1 if (c == 0 and j == 0) else 1
            pend = deque([(0, *emit_qk(0))])
            emit_prev_evac()
            extra = pump_extra[j]
            for t in range(1, last + 1):
                pend.append((t, *emit_qk(t)))
                # per-step PE slack under ACT pacing: exp(t-1)'s duration
                # minus this step's qk and pv matmul time
                w0p = pend[-2][2]
                exp_ns = 2 * (512 - w0p) * 0.833 + 92
                qk_ns = (512 - pend[-1][2]) * 0.417
                slack = exp_ns - qk_ns - 217
                pump(max(0, slack) + extra)
                if len(pend) > lag:
                    tp, et, w0 = pend.popleft()
                    vwait(tp)
                    emit_pv(tp, et, w0)
            pump(pump_extra[j])
            while pend:
                tp, et, w0 = pend.popleft()
                vwait(tp)
                emit_pv(tp, et, w0)

            # --- normalization: deferred into the next slice's stream ---
            rowsums = bass.AP(
                tensor=pv.tensor,
                offset=pv.offset + 64,
                ap=[pv.ap[0], [512, 2], [65, 4]],
            )

            if not final:
                def evac(c=c, j=j, pv=pv, rowsums=rowsums):
                    rcp = rc_p.tile([128, 2, 4], F32, tag="rcp", name="rcp")
                    stage = st_p.tile([128, 512], F16, tag="stage", name="stage")
                    pst = ps_proj.tile([128, 4, 128], F16, tag="ps", name="pst")
                    nc.vector.reciprocal(rcp[:], rowsums)
                    st_v, pv_v, rc_v = evac_views(pv, stage, rcp)
                    nc.vector.tensor_mul(st_v, pv_v, rc_v)
                    for ci in range(4):
                        nc.tensor.transpose(
                            pst[:, ci, :],
                            stage[:, ci * 128 : (ci + 1) * 128],
                            idn_sb[:],
                        )
                    nc.vector.tensor_copy(
                        ot_sb[c][:, j * 512 : (j + 1) * 512],
                        pst[:].rearrange("p a b -> p (a b)"),
                    )
                    if c == 3:
                        low_q.append(((9, 9), (0, 0), gen_oproj(j)))
                prev_evac[0] = evac
            else:
                # final slice: reciprocal first (DVE overlaps the flush),
                # drain fillers (an open filler PSUM group would deadlock
                # pst below), pre-accumulate cc=0..2 for the first s-tile
                # (one open group per pool, keeping a slot free for pst),
                # evacuate pair 3 per 128-chunk, close as chunks land
                rcp = rc_p.tile([128, 2, 4], F32, tag="rcp", name="rcp")
                nc.vector.reciprocal(rcp[:], rowsums)
                flush((9, 9))
                stage = st_p.tile([128, 512], F16, tag="stage", name="stage")
                pst = ps_proj.tile([128, 4, 128], F16, tag="ps", name="pst")
                pos = {}
                for dsl in range(2):
                    pp, ptag = [(ps_proj, "ps"), (ps_qk, "qk")][dsl]
                    pos[dsl] = out_proj(4 * j, dsl, pp, ptag, range(3))
                obs = {}
                for ci in range(4):
                    st_v, pv_v, rc_v = evac_views(pv, stage, rcp, ci)
                    nc.vector.tensor_mul(st_v, pv_v, rc_v)
                    nc.tensor.transpose(
                        pst[:, ci, :],
                        stage[:, ci * 128 : (ci + 1) * 128],
                        idn_sb[:],
                    )
                    nc.vector.tensor_copy(
                        ot_sb[c][:, (4 * j + ci) * 128 : (4 * j + ci + 1) * 128],
                        pst[:, ci, :],
                    )
                    st = 4 * j + ci
                    ob = ob_p.tile([128, D], F16, tag="ob", name="ob")
                    for dsl in range(2):
                        if ci == 0:
                            po = pos[dsl]
                            nc.tensor.matmul(
                                po[:],
                                ot_sb[3][:, st * 128 : (st + 1) * 128],
                                wo_sb[:, 3, dsl * 512 : (dsl + 1) * 512],
                                start=False,
                                stop=True,
                            )
                        else:
                            pp, ptag = [(ps_proj, "ps"), (ps_qk, "qk")][dsl]
                            po = pp.tile([128, 512], F32, tag=ptag, name="po")
                            for cc in range(4):
                                nc.tensor.matmul(
                                    po[:],
                                    ot_sb[cc][:, st * 128 : (st + 1) * 128],
                                    wo_sb[:, cc, dsl * 512 : (dsl + 1) * 512],
                                    start=(cc == 0),
                                    stop=(cc == 3),
                                )
                        nc.vector.tensor_copy(
                            ob[:, dsl * 512 : (dsl + 1) * 512], po[:]
                        )
                        eng = nc.gpsimd if dsl == 0 else nc.sync
                        eng.dma_start(
                            out=out_d[
                                st * 128 : (st + 1) * 128,
                                dsl * 512 : (dsl + 1) * 512,
                            ],
                            in_=ob[:, dsl * 512 : (dsl + 1) * 512],
                        )

        for j in range(ns):
            for c in range(4):
                flush((j, c))
                attention(c, j)
        flush((9, 9))

    if legalize:
        _legalize_waits(nc)
    return nc


_NC_CACHE = {}


def _get_nc(s=S):
    if s not in _NC_CACHE:
        _NC_CACHE[s] = build_nc(s)
    return _NC_CACHE[s]


def make_inputs(X, Wq, bq, Wk, bk, Wv, bv, Wo, bo, s=S):
    """Per-core input maps. Core c: batch c//2, head group c%2."""
    iv, jv = np.arange(128)[:, None], np.arange(128)[None, :]
    mask = (jv >= iv).astype(np.float16)
    idn = np.eye(128, dtype=np.float16)

    def q8(x):
        return np.asarray(x, dtype=ml_dtypes.float8_e4m3fn)

    def lay(a):  # [D, n] -> [128, 4, 2, n]: D = d2*256 + kt*128 + p
        return np.ascontiguousarray(
            a.reshape(4, 2, 128, -1).transpose(2, 0, 1, 3)
        )

    def pack_w(Wt):  # [D, 512] fp32 -> [128, 4, 2, 2, 512] fp8: 32W, 32W-q(32W)
        W8 = q8(32 * Wt)
        return np.stack(
            [lay(W8), lay(q8(32 * Wt - W8.astype(np.float32)))], axis=3
        )

    def pack_w_qk(Wt):
        """[D, 512] fp32 -> [128, 4c, 4d2, 2kt, 128] fp8, W8 only (the Q/K
        projections drop the S8 residual term)."""
        W8 = q8(32 * Wt)  # [D, 512]
        return np.ascontiguousarray(
            W8.reshape(4, 2, 128, 4, 128).transpose(2, 3, 0, 1, 4)
        )

    def pair_perm(a):
        """Permute the trailing 512 group-columns per 128-block from
        [headA hd0-63 | headB hd0-63] to [A-lo | B-lo | A-hi | B-hi] so the
        Q/K projection PSUM partitions match the fp8 DoubleRow pair evac."""
        sh = a.shape[:-1]
        return np.ascontiguousarray(
            a.reshape(*sh, 4, 2, 2, 32).swapaxes(-3, -2).reshape(*sh, 512)
        )

    in_maps = []
    for c in range(8):
        b, g = divmod(c, 2)
        lo, hi = g * GW, (g + 1) * GW
        bqk = 32 * np.concatenate(
            [
                np.ascontiguousarray(pair_perm(bq[lo:hi]).reshape(4, 128).T),
                np.ascontiguousarray(pair_perm(bk[lo:hi]).reshape(4, 128).T),
            ],
            axis=1,
        ).astype(np.float32)
        Xb = np.ascontiguousarray(X[b, :s].T)
        X8 = q8(Xb)
        R8 = q8(Xb - X8.astype(np.float32))
        in_maps.append(
            {
                "xt": np.stack([lay(X8), lay(R8)], axis=3),
                "wq": pack_w_qk(pair_perm(np.ascontiguousarray(Wq[lo:hi].T))),
                "wk": pack_w_qk(pair_perm(np.ascontiguousarray(Wk[lo:hi].T))),
                "wv": pack_w(np.ascontiguousarray(Wv[lo:hi].T)),
                "wo": np.ascontiguousarray(Wo[:, lo:hi].T).astype(np.float16),
                "bqk": bqk,
                "bvb": np.tile(32 * bv[lo:hi], (128, 1)).astype(np.float16),
                "mask": mask,
                "idn": idn,
            }
        )
    return in_maps


def kernel(X, Wq, bq, Wk, bk, Wv, bv, Wo, bo, **run_kwargs):
    args = [np.asarray(a, np.float32) for a in (X, Wq, bq, Wk, bk, Wv, bv, Wo, bo)]
    X, Wq, bq, Wk, bk, Wv, bv, Wo, bo = args
    nc = _get_nc(S)
    in_maps = make_inputs(X, Wq, bq, Wk, bk, Wv, bv, Wo, bo, S)
    res = run_bass_kernel_spmd(nc, in_maps, core_ids=list(range(8)), **run_kwargs)
    outs = [r["out"] for r in res.results]
    full = np.empty((B, S, D), np.float32)
    for b in range(B):
        full[b] = outs[2 * b].astype(np.float32) + outs[2 * b + 1] + bo
    kernel.last_results = res
    return full

